# revision 14
# baseline (speedup 1.0000x reference)
"""Causal multi-head self-attention (b=4, s=2048, d_model=1024, 16 heads) on 8
Trainium2 NeuronCores.

Sharding: core c handles batch c//2 and head-group c%2 (8 of 16 heads):
  - wqkv row-split by head (tensor parallel), wo column-split by head.
  - Each core returns the partial output projection [s, d_model] for its head
    group; the host sums the two partials of each batch while unsharding (the
    pairwise all-reduce of the TP split).

Host-side prep (layout/sharding only): transposes of x/wqkv/wo into the
layouts the PE wants (contraction dim on partitions), per-head permutation of
the Q/K weight rows into [even-features | odd-features] order so RoPE becomes
a rotate-half, and the cos/sin tables from token_positions.

Per-core dataflow (all matmuls float32r = 1 PE cycle/row at free-dim >= 256):
  AB) Fused projections, streaming x^T chunks:
      qkT[f, t] (feature-major, Q then K, head pairs per 128-row tile) with
      RoPE fused:  qk' = cos * qk  +  DMA-swap-add( sin_pm * qk )
      where sin_pm has +sin on lo rows / -sin on hi rows and the DMA-add swaps
      the 32-row halves of each 64-row head block (accum_op=add).
      v[t, f] token-major, stored [t, ktile, head, 65] with a ones column per
      head -- the AV matmul then yields softmax denominators for free.
  C) Attention per (head pair, q-chunk of 512) over causal k-tiles of 128:
      scores^T[k, q]: two concurrent row-tiled matmuls (K=64 each, heads at
      partition halves, tile_position (0,0)/(64,0));
      causal mask: identity @ mtri accumulated onto the diagonal PSUM block;
      exp on ScalarE (PSUM->SBUF, scale=1/8 folded into the activation);
      AV: lhsT = [V_h | 1] (128k x 65) vs expS^T -> PSUM [65, q] accumulated
      over k-tiles; row 64 = softmax denominator per q.
  D) reciprocal of denominators (custom DVE op, ~2 ulp), broadcast across 64
     partitions via a K=1 matmul, normalize y^T on DVE, output projection
     against host-transposed wo columns, partial result DMA'd out.
"""

import sys

if "/opt/trn_rl_repo" not in sys.path:
    sys.path.insert(0, "/opt/trn_rl_repo")

from contextlib import ExitStack

import numpy as np

import concourse.bass as bass  # noqa: F401  (engine types referenced via nc)
import concourse.tile as tile
from concourse import bacc, mybir
from concourse.bass_utils import run_bass_kernel_spmd

F32 = mybir.dt.float32
F32R = mybir.dt.float32r
BF16 = mybir.dt.bfloat16
EXP = mybir.ActivationFunctionType.Exp
MULT = mybir.AluOpType.mult
ADD = mybir.AluOpType.add

# Problem constants
B, S_FULL, D = 4, 2048, 1024
NH_CORE = 8      # heads per core
DH = 64          # head dim
FQK = 1024       # Q+K features per core
FV = 512         # V features per core
P = 128
TCH = 512        # q/t chunk size
NEG = -1.0e30
ROPE_THETA = 10000.0
SCALE = 1.0 / 8.0  # 1/sqrt(DH)

_CACHE = {}


def _emit(nc, tc, S, xT, wqkT, wvT, woT, cosF, sinFpm, mtri, ident, ones2, onesv, outp):
    n_tch = S // TCH
    n_kt = S // P
    n_hp = NH_CORE // 2
    mm = nc.tensor.matmul

    with ExitStack() as ctx:
        # ---------- persistent buffers ----------
        persist = ctx.enter_context(tc.tile_pool(name="persist", bufs=1))
        qkT = [
            persist.tile([P, S], F32R, tag=f"qkT{ft}", name=f"qkT{ft}")
            for ft in range(8)
        ]
        vbuf = persist.tile([P, n_kt, NH_CORE, DH + 1], F32R, tag="vbuf")
        yT = [
            persist.tile([P, S], F32R, tag=f"yT{hp}", name=f"yT{hp}")
            for hp in range(n_hp)
        ]
        # bf16: exact for {0, 1, -1e30}-ish mask values and 1 PE cycle/row even
        # at free-dim 128 (fp32r would be 4 cycles/row below 256)
        ident_sb = persist.tile([P, P], BF16, tag="ident")
        mtri_sb = persist.tile([P, P], BF16, tag="mtri")
        ones2_sb = persist.tile([2, P], F32R, tag="ones2")

        nc.sync.dma_start(ident_sb[:], ident.ap()[:, :])
        nc.sync.dma_start(mtri_sb[:], mtri.ap()[:, :])
        nc.sync.dma_start(ones2_sb[:], ones2.ap()[:, :])
        # memset can't write f32r: fill the V ones-column from DRAM
        nc.sync.dma_start(vbuf[:, :, :, DH : DH + 1], onesv.ap()[:, :, :, :])

        xT_r = xT.ap().rearrange("(eo p) t -> p eo t", p=P)
        wqk_r = wqkT.ap().rearrange("(eo p) f -> p eo f", p=P)

        # ---------- phase A: Q/K projection + fused rope ----------
        with ExitStack() as ab:
            wpool = ab.enter_context(tc.tile_pool(name="wqkft", bufs=3))
            xpool = ab.enter_context(tc.tile_pool(name="xchunk", bufs=2))
            cpool = ab.enter_context(tc.tile_pool(name="costab", bufs=2))
            btpool = ab.enter_context(tc.tile_pool(name="btmp", bufs=2))
            qk_ps = ab.enter_context(
                tc.tile_pool(name="qk_psum", bufs=3, space="PSUM")
            )

            for tci in range(n_tch):
                tsl = slice(tci * TCH, (tci + 1) * TCH)
                xch = xpool.tile([P, 8, TCH], F32R, tag="xch", name="xch")
                nc.sync.dma_start(xch[:], xT_r[:, :, tsl])
                cos_ch = cpool.tile([P, TCH], F32, tag="cos", name="cos")
                sin_ch = cpool.tile([P, TCH], F32, tag="sin", name="sin")
                nc.sync.dma_start(cos_ch[:], cosF.ap()[:, tsl])
                nc.sync.dma_start(sin_ch[:], sinFpm.ap()[:, tsl])

                for ft in range(8):
                    wft = wpool.tile([P, 8, P], F32R, tag="wft", name="wft")
                    nc.sync.dma_start(wft[:], wqk_r[:, :, ft * P : (ft + 1) * P])
                    ps = qk_ps.tile([P, TCH], F32, tag="qkps", name="qkps")
                    for ec in range(8):
                        mm(
                            ps[:],
                            wft[:, ec, :],
                            xch[:, ec, :],
                            start=(ec == 0),
                            stop=(ec == 7),
                        )
                    dst = qkT[ft][:, tsl]
                    nc.vector.tensor_tensor(dst, ps[:], cos_ch[:], MULT)
                    bt = btpool.tile([P, TCH], F32R, tag="bt", name="bt")
                    nc.vector.tensor_tensor(bt[:], ps[:], sin_ch[:], MULT)
                    # swap 32-row halves of each 64-row head block, accumulate
                    for blk in range(4):
                        a = blk * 32
                        c2 = a ^ 32  # partner half within the 64-row block
                        nc.gpsimd.dma_start(
                            dst[c2 : c2 + 32, :], bt[a : a + 32, :], accum_op=ADD
                        )

        # ---------- phase B: V projection (token-major) ----------
        with ExitStack() as bb:
            wvpool = bb.enter_context(tc.tile_pool(name="wvp", bufs=1))
            xpool = bb.enter_context(tc.tile_pool(name="xchunk2", bufs=2))
            v_ps = bb.enter_context(tc.tile_pool(name="v_psum", bufs=2, space="PSUM"))

            wv_sb = wvpool.tile([P, 8, FV], F32R, tag="wv", name="wv")
            nc.sync.dma_start(wv_sb[:], wvT.ap().rearrange("(eo p) f -> p eo f", p=P))
            for tci in range(n_tch):
                tsl = slice(tci * TCH, (tci + 1) * TCH)
                xch = xpool.tile([P, 8, TCH], F32R, tag="xch2", name="xch2")
                nc.sync.dma_start(xch[:], xT_r[:, :, tsl])
                for tti in range(TCH // P):
                    kt = tci * (TCH // P) + tti
                    vps = v_ps.tile([P, FV], F32, tag="vps", name="vps")
                    for ec in range(8):
                        mm(
                            vps[:],
                            xch[:, ec, tti * P : (tti + 1) * P],
                            wv_sb[:, ec, :],
                            start=(ec == 0),
                            stop=(ec == 7),
                        )
                    nc.vector.tensor_copy(vbuf[:, kt, :, 0:DH], vps[:])

        # ---------- phase C: attention (normalization fused in) ----------
        with ExitStack() as c:
            epool = c.enter_context(tc.tile_pool(name="expS", bufs=5))
            dpool = c.enter_context(tc.tile_pool(name="denst", bufs=4))
            rpool = c.enter_context(tc.tile_pool(name="recb", bufs=2))
            s_ps = c.enter_context(tc.tile_pool(name="s_psum", bufs=3, space="PSUM"))
            av_ps = c.enter_context(
                tc.tile_pool(name="av_psum", bufs=4, space="PSUM")
            )
            bc_ps = c.enter_context(
                tc.tile_pool(name="bc_psum", bufs=1, space="PSUM")
            )

            for hp in range(n_hp):
                qt = qkT[hp]
                ktt = qkT[4 + hp]
                h0, h1 = 2 * hp, 2 * hp + 1
                for qci in range(n_tch):
                    qsl = slice(qci * TCH, (qci + 1) * TCH)
                    nkt = (TCH // P) * qci + (TCH // P)
                    avp0 = av_ps.tile([DH + 1, TCH], F32, tag="avp", name="avp0")
                    avp1 = av_ps.tile([DH + 1, TCH], F32, tag="avp", name="avp1")
                    for ki in range(nkt):
                        ksl = slice(ki * P, (ki + 1) * P)
                        diag = ki >= (TCH // P) * qci
                        j = ki - (TCH // P) * qci
                        off = j * P if diag else 0
                        sp0 = s_ps.tile([P, TCH], F32, tag="sp", name="sp0")
                        sp1 = s_ps.tile([P, TCH], F32, tag="sp", name="sp1")
                        mm(sp0[:], ktt[0:64, ksl], qt[0:64, qsl], start=True, stop=True)
                        mm(
                            sp1[:],
                            ktt[64:128, ksl],
                            qt[64:128, qsl],
                            tile_position=(64, 0),
                            start=True,
                            stop=True,
                        )
                        if diag:
                            jsl = slice(j * P, (j + 1) * P)
                            mm(
                                sp0[:, jsl],
                                ident_sb[:],
                                mtri_sb[:],
                                start=False,
                                stop=True,
                                skip_group_check=True,
                            )
                            mm(
                                sp1[:, jsl],
                                ident_sb[:],
                                mtri_sb[:],
                                start=False,
                                stop=True,
                                skip_group_check=True,
                            )
                        e0 = epool.tile([P, TCH], F32R, tag="e0", name="e0")
                        e1 = epool.tile([P, TCH], F32R, tag="e1", name="e1")
                        nc.scalar.activation(e0[:, off:], sp0[:, off:], EXP, scale=SCALE)
                        nc.scalar.activation(e1[:, off:], sp1[:, off:], EXP, scale=SCALE)
                        mm(
                            avp0[:, off:],
                            vbuf[:, ki, h0, :],
                            e0[:, off:],
                            start=(ki == 0),
                            stop=(ki == nkt - 1),
                            skip_group_check=True,
                        )
                        mm(
                            avp1[:, off:],
                            vbuf[:, ki, h1, :],
                            e1[:, off:],
                            start=(ki == 0),
                            stop=(ki == nkt - 1),
                            skip_group_check=True,
                        )
                    # denominators (row 64) -> [1, 512] f32r staging tiles
                    den0 = dpool.tile([1, TCH], F32R, tag="den", name="den0")
                    den1 = dpool.tile([1, TCH], F32R, tag="den", name="den1")
                    nc.scalar.copy(den0[:], avp0[DH : DH + 1, :])
                    nc.scalar.copy(den1[:], avp1[DH : DH + 1, :])
                    den2 = dpool.tile([2, TCH], F32R, tag="den2", name="den2")
                    nc.sync.dma_start(den2[0:1, :], den0[:])
                    nc.sync.dma_start(den2[1:2, :], den1[:])
                    # one K=2 matmul: block-diag ones lhsT broadcasts head-0
                    # denom to partitions 0-63 and head-1 to 64-127
                    rb = bc_ps.tile([P, TCH], F32, tag="rb", name="rb")
                    mm(rb[:], ones2_sb[:, :], den2[:], start=True, stop=True)
                    rec = rpool.tile([P, TCH], F32, tag="rec", name="rec")
                    rscr = rpool.tile([P, TCH], F32, tag="rscr", name="rscr")
                    nc.vector.reciprocal_approx_accurate(rec[:], rb[:], rscr[:])
                    # fused normalize + PSUM->SBUF drain of y^T
                    nc.vector.tensor_tensor(
                        yT[hp][0:64, qsl], avp0[0:DH, :], rec[0:64, :], MULT
                    )
                    nc.vector.tensor_tensor(
                        yT[hp][64:128, qsl], avp1[0:DH, :], rec[64:128, :], MULT
                    )

        # ---------- phase D: output projection ----------
        with ExitStack() as d:
            dpool = d.enter_context(tc.tile_pool(name="dproj", bufs=1))
            opool = d.enter_context(tc.tile_pool(name="outsb", bufs=3))
            o_ps = d.enter_context(tc.tile_pool(name="o_psum", bufs=2, space="PSUM"))

            wo_sb = dpool.tile([P, 4, D], F32R, tag="wo", name="wo")
            nc.sync.dma_start(wo_sb[:], woT.ap().rearrange("(co p) j -> p co j", p=P))
            for tti in range(S // P):
                tsl = slice(tti * P, (tti + 1) * P)
                for jc in range(2):
                    jsl = slice(jc * TCH, (jc + 1) * TCH)
                    op = o_ps.tile([P, TCH], F32, tag="op", name="op")
                    for cc in range(4):
                        mm(
                            op[:],
                            yT[cc][:, tsl],
                            wo_sb[:, cc, jsl],
                            start=(cc == 0),
                            stop=(cc == 3),
                        )
                    ot = opool.tile([P, TCH], F32, tag="ot", name="ot")
                    nc.vector.tensor_copy(ot[:], op[:])
                    nc.sync.dma_start(outp.ap()[tsl, jsl], ot[:])


def _build(S=S_FULL):
    key = ("nc", S)
    if key in _CACHE:
        return _CACHE[key]
    nc = bacc.Bacc("TRN2", target_bir_lowering=False, debug=False, num_devices=8)
    xT = nc.dram_tensor("xT", [D, S], F32R, kind="ExternalInput")
    wqkT = nc.dram_tensor("wqkT", [D, FQK], F32R, kind="ExternalInput")
    wvT = nc.dram_tensor("wvT", [D, FV], F32R, kind="ExternalInput")
    woT = nc.dram_tensor("woT", [FV, D], F32R, kind="ExternalInput")
    cosF = nc.dram_tensor("cosF", [P, S], F32, kind="ExternalInput")
    sinFpm = nc.dram_tensor("sinFpm", [P, S], F32, kind="ExternalInput")
    mtri = nc.dram_tensor("mtri", [P, P], BF16, kind="ExternalInput")
    ident = nc.dram_tensor("ident", [P, P], BF16, kind="ExternalInput")
    ones2 = nc.dram_tensor("ones2", [2, P], F32R, kind="ExternalInput")
    onesv = nc.dram_tensor(
        "onesv", [P, S // P, NH_CORE, 1], F32R, kind="ExternalInput"
    )
    outp = nc.dram_tensor("outp", [S, D], F32, kind="ExternalOutput")
    with tile.TileContext(nc) as tc:
        _emit(nc, tc, S, xT, wqkT, wvT, woT, cosF, sinFpm, mtri, ident, ones2, onesv, outp)
    nc.compile()
    _CACHE[key] = nc
    return nc


def host_inputs(x, wqkv, wo, token_positions, S=S_FULL):
    """Build the 8 per-core input maps (host-side sharding / layout prep)."""
    x = np.asarray(x, dtype=np.float32)
    wqkv = np.asarray(wqkv, dtype=np.float32)
    wo = np.asarray(wo, dtype=np.float32)
    pos = np.asarray(token_positions).astype(np.float32)

    d_model = x.shape[2]
    wq, wk, wv = wqkv[0:d_model], wqkv[d_model : 2 * d_model], wqkv[2 * d_model :]

    inv = np.float32(ROPE_THETA) ** (
        -np.arange(0, DH, 2, dtype=np.float32) / np.float32(DH)
    )  # [32]
    ang = pos[None, :] * inv[:, None]  # [32, S]
    cos32 = np.cos(ang).astype(np.float32)
    sin32 = np.sin(ang).astype(np.float32)
    cosF = np.tile(cos32, (4, 1))  # [128, S]
    sinFpm = np.tile(np.concatenate([sin32, -sin32], axis=0), (2, 1))  # [128, S]

    import ml_dtypes

    a = np.arange(P)
    mtri = np.where(a[:, None] > a[None, :], np.float32(NEG), np.float32(0.0))
    mtri = mtri.astype(ml_dtypes.bfloat16)
    ident = np.eye(P, dtype=ml_dtypes.bfloat16)
    S = x.shape[1]
    ones2 = np.zeros((2, P), np.float32)
    ones2[0, 0:64] = 1.0
    ones2[1, 64:128] = 1.0
    onesv = np.ones((P, S // P, NH_CORE, 1), np.float32)

    perm64 = np.concatenate([np.arange(0, DH, 2), np.arange(1, DH, 2)])

    in_maps = []
    for ci in range(8):
        bi, hg = divmod(ci, 2)
        xT = np.ascontiguousarray(x[bi].T)
        rows = []
        for blk in (wq, wk):
            for h in range(hg * NH_CORE, (hg + 1) * NH_CORE):
                rows.append(blk[h * DH : (h + 1) * DH][perm64])
        wqkT = np.ascontiguousarray(np.concatenate(rows, axis=0).T)
        wvT = np.ascontiguousarray(wv[hg * FV : (hg + 1) * FV].T)
        woT = np.ascontiguousarray(wo[:, hg * FV : (hg + 1) * FV].T)
        in_maps.append(
            {
                "xT": xT,
                "wqkT": wqkT,
                "wvT": wvT,
                "woT": woT,
                "cosF": cosF,
                "sinFpm": sinFpm,
                "mtri": mtri,
                "ident": ident,
                "ones2": ones2,
                "onesv": onesv,
            }
        )
    return in_maps


def _install_ntff_hook():
    """Recreate the antenv.axon_hooks NTFF profile hook this image lacks
    (same ctypes shim trn_agent_boot would register). Dev/profiling only."""
    import contextlib
    import ctypes
    import os
    import types

    try:
        import antenv.axon_hooks  # noqa: F401

        return
    except ImportError:
        pass
    so_path = "/opt/axon/libaxon_pjrt.so"
    if not os.path.exists(so_path):
        return
    lib = ctypes.CDLL(so_path)
    if not hasattr(lib, "axon_start_nrt_profile"):
        return
    lib.axon_start_nrt_profile.argtypes = [
        ctypes.POINTER(ctypes.c_int64),
        ctypes.c_size_t,
    ]
    lib.axon_start_nrt_profile.restype = ctypes.c_int64
    lib.axon_stop_nrt_profile.argtypes = [ctypes.c_char_p]
    lib.axon_stop_nrt_profile.restype = ctypes.c_int64

    @contextlib.contextmanager
    def _hook(output_dir, device_ids):
        import jax

        jax.devices()
        if device_ids:
            ids = (ctypes.c_int64 * len(device_ids))(*device_ids)
            rc = lib.axon_start_nrt_profile(ids, len(device_ids))
        else:
            rc = lib.axon_start_nrt_profile(None, 0)
        if rc != 0:
            raise RuntimeError(f"axon_start_nrt_profile rc={rc}")
        try:
            yield
        finally:
            n = lib.axon_stop_nrt_profile(str(output_dir).encode())
            if n < 0:
                raise RuntimeError(f"axon_stop_nrt_profile rc={n}")

    import antenv
    from concourse import bass_utils as _bu

    _bu.upload_artifacts = lambda d: d  # no bucket access in this container
    mod = types.ModuleType("antenv.axon_hooks")
    mod.get_axon_ntff_profile_hook = lambda: _hook
    mod.set_axon_ntff_profile_hook = lambda h: None
    sys.modules["antenv.axon_hooks"] = mod
    antenv.axon_hooks = mod


def kernel(x, wqkv, wo, token_positions, trace=False):
    if trace:
        _install_ntff_hook()
    nc = _build()
    in_maps = host_inputs(x, wqkv, wo, token_positions)
    res = run_bass_kernel_spmd(nc, in_maps, core_ids=list(range(8)), trace=trace)
    parts = [res.results[ci]["outp"] for ci in range(8)]
    out = np.stack([parts[2 * bi] + parts[2 * bi + 1] for bi in range(B)], axis=0)
    if trace:
        kernel.last_result = res
    return out


# revision 15
# speedup vs baseline: 1.3655x; 1.3655x over previous
"""Causal multi-head self-attention (b=4, s=2048, d_model=1024, 16 heads) on 8
Trainium2 NeuronCores.

Sharding: core c handles batch c//2 and head-group c%2 (8 of 16 heads):
  - wqkv row-split by head (tensor parallel), wo column-split by head.
  - Each core returns the partial output projection [s, d_model] for its head
    group; the host sums the two partials of each batch while unsharding (the
    pairwise all-reduce of the TP split).

Host-side prep (layout/sharding only): transposes of x/wqkv/wo into the
layouts the PE wants (contraction dim on partitions), per-head permutation of
the Q/K weight rows into [even-features | odd-features] order so RoPE becomes
a rotate-half, and the cos/sin tables from token_positions.

Per-core dataflow (all matmuls float32r = 1 PE cycle/row at free-dim >= 256):
  AB) Fused projections, streaming x^T chunks:
      qkT[f, t] (feature-major, Q then K, head pairs per 128-row tile) with
      RoPE fused:  qk' = cos * qk  +  DMA-swap-add( sin_pm * qk )
      where sin_pm has +sin on lo rows / -sin on hi rows and the DMA-add swaps
      the 32-row halves of each 64-row head block (accum_op=add).
      v[t, f] token-major, stored [t, ktile, head, 65] with a ones column per
      head -- the AV matmul then yields softmax denominators for free.
  C) Attention per (head pair, q-chunk of 512) over causal k-tiles of 128:
      scores^T[k, q]: two concurrent row-tiled matmuls (K=64 each, heads at
      partition halves, tile_position (0,0)/(64,0));
      causal mask: identity @ mtri accumulated onto the diagonal PSUM block;
      exp on ScalarE (PSUM->SBUF, scale=1/8 folded into the activation);
      AV: lhsT = [V_h | 1] (128k x 65) vs expS^T -> PSUM [65, q] accumulated
      over k-tiles; row 64 = softmax denominator per q.
  D) reciprocal of denominators (custom DVE op, ~2 ulp), broadcast across 64
     partitions via a K=1 matmul, normalize y^T on DVE, output projection
     against host-transposed wo columns, partial result DMA'd out.
"""

import sys

if "/opt/trn_rl_repo" not in sys.path:
    sys.path.insert(0, "/opt/trn_rl_repo")

from contextlib import ExitStack

import numpy as np

import concourse.bass as bass  # noqa: F401  (engine types referenced via nc)
import concourse.tile as tile
from concourse import bacc, mybir
from concourse.bass_utils import run_bass_kernel_spmd

F32 = mybir.dt.float32
F32R = mybir.dt.float32r
BF16 = mybir.dt.bfloat16
EXP = mybir.ActivationFunctionType.Exp
MULT = mybir.AluOpType.mult
ADD = mybir.AluOpType.add

# Problem constants
B, S_FULL, D = 4, 2048, 1024
NH_CORE = 8      # heads per core
DH = 64          # head dim
FQK = 1024       # Q+K features per core
FV = 512         # V features per core
P = 128
TCH = 512        # q/t chunk size
NEG = -1.0e30
ROPE_THETA = 10000.0
SCALE = 1.0 / 8.0  # 1/sqrt(DH)

_CACHE = {}


def _emit(nc, tc, S, xT, wqkT, wvT, woT, cosF, sinFpm, mtri, ident, ones2, onesv, outp):
    n_tch = S // TCH
    n_kt = S // P
    n_hp = NH_CORE // 2
    mm = nc.tensor.matmul

    with ExitStack() as ctx:
        # ---------- persistent buffers ----------
        persist = ctx.enter_context(tc.tile_pool(name="persist", bufs=1))
        qkT = [
            persist.tile([P, S], BF16, tag=f"qkT{ft}", name=f"qkT{ft}")
            for ft in range(8)
        ]
        vbuf = persist.tile([P, n_kt, NH_CORE, DH + 1], BF16, tag="vbuf")
        yT = [
            persist.tile([P, S], BF16, tag=f"yT{hp}", name=f"yT{hp}")
            for hp in range(n_hp)
        ]
        # bf16: exact for {0, 1, -1e30}-ish mask values and 1 PE cycle/row even
        # at free-dim 128 (fp32r would be 4 cycles/row below 256)
        ident_sb = persist.tile([P, P], BF16, tag="ident")
        mtri_sb = persist.tile([P, P], BF16, tag="mtri")
        ones2_sb = persist.tile([2, P], F32R, tag="ones2")

        nc.sync.dma_start(ident_sb[:], ident.ap()[:, :])
        nc.sync.dma_start(mtri_sb[:], mtri.ap()[:, :])
        nc.sync.dma_start(ones2_sb[:], ones2.ap()[:, :])
        # memset can't write f32r: fill the V ones-column from DRAM
        nc.sync.dma_start(vbuf[:, :, :, DH : DH + 1], onesv.ap()[:, :, :, :])

        xT_r = xT.ap().rearrange("(eo p) t -> p eo t", p=P)
        wqk_r = wqkT.ap().rearrange("(eo p) f -> p eo f", p=P)

        # ---------- phase A: Q/K projection + fused rope ----------
        with ExitStack() as ab:
            wpool = ab.enter_context(tc.tile_pool(name="wqkft", bufs=3))
            xpool = ab.enter_context(tc.tile_pool(name="xchunk", bufs=2))
            cpool = ab.enter_context(tc.tile_pool(name="costab", bufs=2))
            btpool = ab.enter_context(tc.tile_pool(name="btmp", bufs=2))
            qk_ps = ab.enter_context(
                tc.tile_pool(name="qk_psum", bufs=3, space="PSUM")
            )

            for tci in range(n_tch):
                tsl = slice(tci * TCH, (tci + 1) * TCH)
                xch = xpool.tile([P, 8, TCH], BF16, tag="xch", name="xch")
                nc.sync.dma_start(xch[:], xT_r[:, :, tsl])
                cos_ch = cpool.tile([P, TCH], F32, tag="cos", name="cos")
                sin_ch = cpool.tile([P, TCH], F32, tag="sin", name="sin")
                nc.sync.dma_start(cos_ch[:], cosF.ap()[:, tsl])
                nc.sync.dma_start(sin_ch[:], sinFpm.ap()[:, tsl])

                for ft in range(8):
                    wft = wpool.tile([P, 8, P], BF16, tag="wft", name="wft")
                    nc.sync.dma_start(wft[:], wqk_r[:, :, ft * P : (ft + 1) * P])
                    ps = qk_ps.tile([P, TCH], F32, tag="qkps", name="qkps")
                    for ec in range(8):
                        mm(
                            ps[:],
                            wft[:, ec, :],
                            xch[:, ec, :],
                            start=(ec == 0),
                            stop=(ec == 7),
                        )
                    dst = qkT[ft][:, tsl]
                    nc.vector.tensor_tensor(dst, ps[:], cos_ch[:], MULT)
                    bt = btpool.tile([P, TCH], BF16, tag="bt", name="bt")
                    nc.vector.tensor_tensor(bt[:], ps[:], sin_ch[:], MULT)
                    # swap 32-row halves of each 64-row head block, accumulate
                    for blk in range(4):
                        a = blk * 32
                        c2 = a ^ 32  # partner half within the 64-row block
                        nc.gpsimd.dma_start(
                            dst[c2 : c2 + 32, :], bt[a : a + 32, :], accum_op=ADD
                        )

        # ---------- phase B: V projection (token-major) ----------
        with ExitStack() as bb:
            wvpool = bb.enter_context(tc.tile_pool(name="wvp", bufs=1))
            xpool = bb.enter_context(tc.tile_pool(name="xchunk2", bufs=2))
            v_ps = bb.enter_context(tc.tile_pool(name="v_psum", bufs=2, space="PSUM"))

            wv_sb = wvpool.tile([P, 8, FV], BF16, tag="wv", name="wv")
            nc.sync.dma_start(wv_sb[:], wvT.ap().rearrange("(eo p) f -> p eo f", p=P))
            for tci in range(n_tch):
                tsl = slice(tci * TCH, (tci + 1) * TCH)
                xch = xpool.tile([P, 8, TCH], BF16, tag="xch2", name="xch2")
                nc.sync.dma_start(xch[:], xT_r[:, :, tsl])
                for tti in range(TCH // P):
                    kt = tci * (TCH // P) + tti
                    vps = v_ps.tile([P, FV], F32, tag="vps", name="vps")
                    for ec in range(8):
                        mm(
                            vps[:],
                            xch[:, ec, tti * P : (tti + 1) * P],
                            wv_sb[:, ec, :],
                            start=(ec == 0),
                            stop=(ec == 7),
                        )
                    nc.vector.tensor_copy(vbuf[:, kt, :, 0:DH], vps[:])

        # ---------- phase C: attention (normalization fused in) ----------
        with ExitStack() as c:
            epool = c.enter_context(tc.tile_pool(name="expS", bufs=5))
            dpool = c.enter_context(tc.tile_pool(name="denst", bufs=4))
            rpool = c.enter_context(tc.tile_pool(name="recb", bufs=2))
            s_ps = c.enter_context(tc.tile_pool(name="s_psum", bufs=3, space="PSUM"))
            av_ps = c.enter_context(
                tc.tile_pool(name="av_psum", bufs=4, space="PSUM")
            )
            bc_ps = c.enter_context(
                tc.tile_pool(name="bc_psum", bufs=1, space="PSUM")
            )

            for hp in range(n_hp):
                qt = qkT[hp]
                ktt = qkT[4 + hp]
                h0, h1 = 2 * hp, 2 * hp + 1
                for qci in range(n_tch):
                    qsl = slice(qci * TCH, (qci + 1) * TCH)
                    nkt = (TCH // P) * qci + (TCH // P)
                    avp0 = av_ps.tile([DH + 1, TCH], F32, tag="avp", name="avp0")
                    avp1 = av_ps.tile([DH + 1, TCH], F32, tag="avp", name="avp1")
                    for ki in range(nkt):
                        ksl = slice(ki * P, (ki + 1) * P)
                        diag = ki >= (TCH // P) * qci
                        j = ki - (TCH // P) * qci
                        off = j * P if diag else 0
                        sp0 = s_ps.tile([P, TCH], F32, tag="sp", name="sp0")
                        sp1 = s_ps.tile([P, TCH], F32, tag="sp", name="sp1")
                        mm(sp0[:], ktt[0:64, ksl], qt[0:64, qsl], start=True, stop=True)
                        mm(
                            sp1[:],
                            ktt[64:128, ksl],
                            qt[64:128, qsl],
                            tile_position=(64, 0),
                            start=True,
                            stop=True,
                        )
                        if diag:
                            jsl = slice(j * P, (j + 1) * P)
                            mm(
                                sp0[:, jsl],
                                ident_sb[:],
                                mtri_sb[:],
                                start=False,
                                stop=True,
                                skip_group_check=True,
                            )
                            mm(
                                sp1[:, jsl],
                                ident_sb[:],
                                mtri_sb[:],
                                start=False,
                                stop=True,
                                skip_group_check=True,
                            )
                        e0 = epool.tile([P, TCH], BF16, tag="e0", name="e0")
                        e1 = epool.tile([P, TCH], BF16, tag="e1", name="e1")
                        nc.scalar.activation(e0[:, off:], sp0[:, off:], EXP, scale=SCALE)
                        nc.scalar.activation(e1[:, off:], sp1[:, off:], EXP, scale=SCALE)
                        mm(
                            avp0[:, off:],
                            vbuf[:, ki, h0, :],
                            e0[:, off:],
                            start=(ki == 0),
                            stop=(ki == nkt - 1),
                            skip_group_check=True,
                        )
                        mm(
                            avp1[:, off:],
                            vbuf[:, ki, h1, :],
                            e1[:, off:],
                            start=(ki == 0),
                            stop=(ki == nkt - 1),
                            skip_group_check=True,
                        )
                    # denominators (row 64) -> [1, 512] f32r staging tiles
                    den0 = dpool.tile([1, TCH], F32R, tag="den", name="den0")
                    den1 = dpool.tile([1, TCH], F32R, tag="den", name="den1")
                    nc.scalar.copy(den0[:], avp0[DH : DH + 1, :])
                    nc.scalar.copy(den1[:], avp1[DH : DH + 1, :])
                    den2 = dpool.tile([2, TCH], F32R, tag="den2", name="den2")
                    nc.sync.dma_start(den2[0:1, :], den0[:])
                    nc.sync.dma_start(den2[1:2, :], den1[:])
                    # one K=2 matmul: block-diag ones lhsT broadcasts head-0
                    # denom to partitions 0-63 and head-1 to 64-127
                    rb = bc_ps.tile([P, TCH], F32, tag="rb", name="rb")
                    mm(rb[:], ones2_sb[:, :], den2[:], start=True, stop=True)
                    rec = rpool.tile([P, TCH], F32, tag="rec", name="rec")
                    rscr = rpool.tile([P, TCH], F32, tag="rscr", name="rscr")
                    nc.vector.reciprocal_approx_accurate(rec[:], rb[:], rscr[:])
                    # fused normalize + PSUM->SBUF drain of y^T
                    nc.vector.tensor_tensor(
                        yT[hp][0:64, qsl], avp0[0:DH, :], rec[0:64, :], MULT
                    )
                    nc.vector.tensor_tensor(
                        yT[hp][64:128, qsl], avp1[0:DH, :], rec[64:128, :], MULT
                    )

        # ---------- phase D: output projection ----------
        with ExitStack() as d:
            dpool = d.enter_context(tc.tile_pool(name="dproj", bufs=1))
            opool = d.enter_context(tc.tile_pool(name="outsb", bufs=3))
            o_ps = d.enter_context(tc.tile_pool(name="o_psum", bufs=2, space="PSUM"))

            wo_sb = dpool.tile([P, 4, D], BF16, tag="wo", name="wo")
            nc.sync.dma_start(wo_sb[:], woT.ap().rearrange("(co p) j -> p co j", p=P))
            for tti in range(S // P):
                tsl = slice(tti * P, (tti + 1) * P)
                for jc in range(2):
                    jsl = slice(jc * TCH, (jc + 1) * TCH)
                    op = o_ps.tile([P, TCH], F32, tag="op", name="op")
                    for cc in range(4):
                        mm(
                            op[:],
                            yT[cc][:, tsl],
                            wo_sb[:, cc, jsl],
                            start=(cc == 0),
                            stop=(cc == 3),
                        )
                    ot = opool.tile([P, TCH], F32, tag="ot", name="ot")
                    nc.vector.tensor_copy(ot[:], op[:])
                    nc.sync.dma_start(outp.ap()[tsl, jsl], ot[:])


def _build(S=S_FULL):
    key = ("nc", S)
    if key in _CACHE:
        return _CACHE[key]
    nc = bacc.Bacc("TRN2", target_bir_lowering=False, debug=False, num_devices=8)
    xT = nc.dram_tensor("xT", [D, S], BF16, kind="ExternalInput")
    wqkT = nc.dram_tensor("wqkT", [D, FQK], BF16, kind="ExternalInput")
    wvT = nc.dram_tensor("wvT", [D, FV], BF16, kind="ExternalInput")
    woT = nc.dram_tensor("woT", [FV, D], BF16, kind="ExternalInput")
    cosF = nc.dram_tensor("cosF", [P, S], F32, kind="ExternalInput")
    sinFpm = nc.dram_tensor("sinFpm", [P, S], F32, kind="ExternalInput")
    mtri = nc.dram_tensor("mtri", [P, P], BF16, kind="ExternalInput")
    ident = nc.dram_tensor("ident", [P, P], BF16, kind="ExternalInput")
    ones2 = nc.dram_tensor("ones2", [2, P], F32R, kind="ExternalInput")
    onesv = nc.dram_tensor(
        "onesv", [P, S // P, NH_CORE, 1], BF16, kind="ExternalInput"
    )
    outp = nc.dram_tensor("outp", [S, D], F32, kind="ExternalOutput")
    with tile.TileContext(nc) as tc:
        _emit(nc, tc, S, xT, wqkT, wvT, woT, cosF, sinFpm, mtri, ident, ones2, onesv, outp)
    nc.compile()
    _CACHE[key] = nc
    return nc


def host_inputs(x, wqkv, wo, token_positions, S=S_FULL):
    """Build the 8 per-core input maps (host-side sharding / layout prep)."""
    x = np.asarray(x, dtype=np.float32)
    wqkv = np.asarray(wqkv, dtype=np.float32)
    wo = np.asarray(wo, dtype=np.float32)
    pos = np.asarray(token_positions).astype(np.float32)

    d_model = x.shape[2]
    wq, wk, wv = wqkv[0:d_model], wqkv[d_model : 2 * d_model], wqkv[2 * d_model :]

    inv = np.float32(ROPE_THETA) ** (
        -np.arange(0, DH, 2, dtype=np.float32) / np.float32(DH)
    )  # [32]
    ang = pos[None, :] * inv[:, None]  # [32, S]
    cos32 = np.cos(ang).astype(np.float32)
    sin32 = np.sin(ang).astype(np.float32)
    cosF = np.tile(cos32, (4, 1))  # [128, S]
    sinFpm = np.tile(np.concatenate([sin32, -sin32], axis=0), (2, 1))  # [128, S]

    import ml_dtypes

    a = np.arange(P)
    mtri = np.where(a[:, None] > a[None, :], np.float32(NEG), np.float32(0.0))
    mtri = mtri.astype(ml_dtypes.bfloat16)
    ident = np.eye(P, dtype=ml_dtypes.bfloat16)
    S = x.shape[1]
    ones2 = np.zeros((2, P), np.float32)
    ones2[0, 0:64] = 1.0
    ones2[1, 64:128] = 1.0
    onesv = np.ones((P, S // P, NH_CORE, 1), ml_dtypes.bfloat16)

    perm64 = np.concatenate([np.arange(0, DH, 2), np.arange(1, DH, 2)])

    in_maps = []
    for ci in range(8):
        bi, hg = divmod(ci, 2)
        xT = np.ascontiguousarray(x[bi].T)
        rows = []
        for blk in (wq, wk):
            for h in range(hg * NH_CORE, (hg + 1) * NH_CORE):
                rows.append(blk[h * DH : (h + 1) * DH][perm64])
        wqkT = np.ascontiguousarray(np.concatenate(rows, axis=0).T)
        wvT = np.ascontiguousarray(wv[hg * FV : (hg + 1) * FV].T)
        woT = np.ascontiguousarray(wo[:, hg * FV : (hg + 1) * FV].T)
        xT = xT.astype(ml_dtypes.bfloat16)
        wqkT = wqkT.astype(ml_dtypes.bfloat16)
        wvT = wvT.astype(ml_dtypes.bfloat16)
        woT = woT.astype(ml_dtypes.bfloat16)
        in_maps.append(
            {
                "xT": xT,
                "wqkT": wqkT,
                "wvT": wvT,
                "woT": woT,
                "cosF": cosF,
                "sinFpm": sinFpm,
                "mtri": mtri,
                "ident": ident,
                "ones2": ones2,
                "onesv": onesv,
            }
        )
    return in_maps


def _install_ntff_hook():
    """Recreate the antenv.axon_hooks NTFF profile hook this image lacks
    (same ctypes shim trn_agent_boot would register). Dev/profiling only."""
    import contextlib
    import ctypes
    import os
    import types

    try:
        import antenv.axon_hooks  # noqa: F401

        return
    except ImportError:
        pass
    so_path = "/opt/axon/libaxon_pjrt.so"
    if not os.path.exists(so_path):
        return
    lib = ctypes.CDLL(so_path)
    if not hasattr(lib, "axon_start_nrt_profile"):
        return
    lib.axon_start_nrt_profile.argtypes = [
        ctypes.POINTER(ctypes.c_int64),
        ctypes.c_size_t,
    ]
    lib.axon_start_nrt_profile.restype = ctypes.c_int64
    lib.axon_stop_nrt_profile.argtypes = [ctypes.c_char_p]
    lib.axon_stop_nrt_profile.restype = ctypes.c_int64

    @contextlib.contextmanager
    def _hook(output_dir, device_ids):
        import jax

        jax.devices()
        if device_ids:
            ids = (ctypes.c_int64 * len(device_ids))(*device_ids)
            rc = lib.axon_start_nrt_profile(ids, len(device_ids))
        else:
            rc = lib.axon_start_nrt_profile(None, 0)
        if rc != 0:
            raise RuntimeError(f"axon_start_nrt_profile rc={rc}")
        try:
            yield
        finally:
            n = lib.axon_stop_nrt_profile(str(output_dir).encode())
            if n < 0:
                raise RuntimeError(f"axon_stop_nrt_profile rc={n}")

    import antenv
    from concourse import bass_utils as _bu

    _bu.upload_artifacts = lambda d: d  # no bucket access in this container
    mod = types.ModuleType("antenv.axon_hooks")
    mod.get_axon_ntff_profile_hook = lambda: _hook
    mod.set_axon_ntff_profile_hook = lambda h: None
    sys.modules["antenv.axon_hooks"] = mod
    antenv.axon_hooks = mod


def kernel(x, wqkv, wo, token_positions, trace=False):
    if trace:
        _install_ntff_hook()
    nc = _build()
    in_maps = host_inputs(x, wqkv, wo, token_positions)
    res = run_bass_kernel_spmd(nc, in_maps, core_ids=list(range(8)), trace=trace)
    parts = [res.results[ci]["outp"] for ci in range(8)]
    out = np.stack([parts[2 * bi] + parts[2 * bi + 1] for bi in range(B)], axis=0)
    if trace:
        kernel.last_result = res
    return out


# revision 17
# speedup vs baseline: 1.7939x; 1.3138x over previous
"""Causal multi-head self-attention (b=4, s=2048, d_model=1024, 16 heads) on 8
Trainium2 NeuronCores.

Sharding: core c handles batch c//2 and head-group c%2 (8 of 16 heads):
  - wqkv row-split by head (tensor parallel), wo column-split by head.
  - Each core returns the partial output projection [s, d_model] for its head
    group; the host sums the two partials of each batch while unsharding (the
    pairwise all-reduce of the TP split).

Host-side prep (layout/sharding only): transposes of x/wqkv/wo into the
layouts the PE wants (contraction dim on partitions), per-head permutation of
the Q/K weight rows into [even-features | odd-features] order so RoPE becomes
a rotate-half, and the cos/sin tables from token_positions.

Per-core dataflow (all matmuls float32r = 1 PE cycle/row at free-dim >= 256):
  AB) Fused projections, streaming x^T chunks:
      qkT[f, t] (feature-major, Q then K, head pairs per 128-row tile) with
      RoPE fused:  qk' = cos * qk  +  DMA-swap-add( sin_pm * qk )
      where sin_pm has +sin on lo rows / -sin on hi rows and the DMA-add swaps
      the 32-row halves of each 64-row head block (accum_op=add).
      v[t, f] token-major, stored [t, ktile, head, 65] with a ones column per
      head -- the AV matmul then yields softmax denominators for free.
  C) Attention per (head pair, q-chunk of 512) over causal k-tiles of 128:
      scores^T[k, q]: two concurrent row-tiled matmuls (K=64 each, heads at
      partition halves, tile_position (0,0)/(64,0));
      causal mask: identity @ mtri accumulated onto the diagonal PSUM block;
      exp on ScalarE (PSUM->SBUF, scale=1/8 folded into the activation);
      AV: lhsT = [V_h | 1] (128k x 65) vs expS^T -> PSUM [65, q] accumulated
      over k-tiles; row 64 = softmax denominator per q.
  D) reciprocal of denominators (custom DVE op, ~2 ulp), broadcast across 64
     partitions via a K=1 matmul, normalize y^T on DVE, output projection
     against host-transposed wo columns, partial result DMA'd out.
"""

import sys

if "/opt/trn_rl_repo" not in sys.path:
    sys.path.insert(0, "/opt/trn_rl_repo")

from contextlib import ExitStack

import numpy as np

import concourse.bass as bass  # noqa: F401  (engine types referenced via nc)
import concourse.tile as tile
from concourse import bacc, mybir
from concourse.bass_utils import run_bass_kernel_spmd

F32 = mybir.dt.float32
F32R = mybir.dt.float32r
BF16 = mybir.dt.bfloat16
EXP = mybir.ActivationFunctionType.Exp
MULT = mybir.AluOpType.mult
ADD = mybir.AluOpType.add

# Problem constants
B, S_FULL, D = 4, 2048, 1024
NH_CORE = 8      # heads per core
DH = 64          # head dim
FQK = 1024       # Q+K features per core
FV = 512         # V features per core
P = 128
TCH = 512        # q/t chunk size
NEG = -1.0e30
ROPE_THETA = 10000.0
SCALE = 1.0 / 8.0  # 1/sqrt(DH)

_CACHE = {}


def _emit(nc, tc, S, xT, wqkT, wvT, woT, cosF, sinFpm, mtri, ident, ones2, onesv, outp):
    n_tch = S // TCH
    n_kt = S // P
    n_hp = NH_CORE // 2
    mm = nc.tensor.matmul

    with ExitStack() as ctx:
        # ---------- persistent buffers ----------
        persist = ctx.enter_context(tc.tile_pool(name="persist", bufs=1))
        qkT = [
            persist.tile([P, S], BF16, tag=f"qkT{ft}", name=f"qkT{ft}")
            for ft in range(8)
        ]
        vbuf = persist.tile([P, n_kt, NH_CORE, DH + 1], BF16, tag="vbuf")
        yT = [
            persist.tile([P, S], BF16, tag=f"yT{hp}", name=f"yT{hp}")
            for hp in range(n_hp)
        ]
        ident_sb = persist.tile([P, P], BF16, tag="ident")
        mtri_sb = persist.tile([P, P], BF16, tag="mtri")
        ones2_sb = persist.tile([2, P], F32R, tag="ones2")

        nc.sync.dma_start(ident_sb[:], ident.ap()[:, :])
        nc.sync.dma_start(mtri_sb[:], mtri.ap()[:, :])
        nc.sync.dma_start(ones2_sb[:], ones2.ap()[:, :])
        nc.sync.dma_start(vbuf[:, :, :, DH : DH + 1], onesv.ap()[:, :, :, :])

        xT_r = xT.ap().rearrange("(eo p) t -> p eo t", p=P)
        wqk_r = wqkT.ap().rearrange("(eo p) f -> p eo f", p=P)

        # ---------- phase B first: V projection (token-major) ----------
        with ExitStack() as bb:
            wvpool = bb.enter_context(tc.tile_pool(name="wvp", bufs=1))
            xpool = bb.enter_context(tc.tile_pool(name="xchunk2", bufs=2))
            v_ps = bb.enter_context(tc.tile_pool(name="v_psum", bufs=2, space="PSUM"))

            wv_sb = wvpool.tile([P, 8, FV], BF16, tag="wv", name="wv")
            nc.sync.dma_start(wv_sb[:], wvT.ap().rearrange("(eo p) f -> p eo f", p=P))
            for tci in range(n_tch):
                tsl = slice(tci * TCH, (tci + 1) * TCH)
                xch = xpool.tile([P, 8, TCH], BF16, tag="xch2", name="xch2")
                nc.sync.dma_start(xch[:], xT_r[:, :, tsl])
                for tti in range(TCH // P):
                    kt = tci * (TCH // P) + tti
                    vps = v_ps.tile([P, FV], F32, tag="vps", name="vps")
                    for ec in range(8):
                        mm(
                            vps[:],
                            xch[:, ec, tti * P : (tti + 1) * P],
                            wv_sb[:, ec, :],
                            start=(ec == 0),
                            stop=(ec == 7),
                        )
                    nc.vector.tensor_copy(vbuf[:, kt, :, 0:DH], vps[:])

        # ---------- phases A (per head pair) and C, emitted interleaved so
        # the scheduler overlaps C's exp-bound stretches with A's matmuls ----
        ac = ctx.enter_context(ExitStack())
        apool = ac.enter_context(tc.tile_pool(name="wqkft", bufs=3))
        xpool = ac.enter_context(tc.tile_pool(name="xchunk", bufs=2))
        cpool = ac.enter_context(tc.tile_pool(name="costab", bufs=2))
        btpool = ac.enter_context(tc.tile_pool(name="btmp", bufs=2))
        qk_ps = ac.enter_context(tc.tile_pool(name="qk_psum", bufs=1, space="PSUM"))
        epool = ac.enter_context(tc.tile_pool(name="expS", bufs=5))
        dpool = ac.enter_context(tc.tile_pool(name="denst", bufs=4))
        rpool = ac.enter_context(tc.tile_pool(name="recb", bufs=2))
        s_ps = ac.enter_context(tc.tile_pool(name="s_psum", bufs=2, space="PSUM"))
        av_ps = ac.enter_context(tc.tile_pool(name="av_psum", bufs=1, space="PSUM"))
        bc_ps = ac.enter_context(tc.tile_pool(name="bc_psum", bufs=1, space="PSUM"))

        def phase_a(hp):
            # Q/K projection + fused rope for f-tiles hp (Q) and 4+hp (K)
            for tci in range(n_tch):
                tsl = slice(tci * TCH, (tci + 1) * TCH)
                xch = xpool.tile([P, 8, TCH], BF16, tag="xch", name="xch")
                nc.sync.dma_start(xch[:], xT_r[:, :, tsl])
                cos_ch = cpool.tile([P, TCH], F32, tag="cos", name="cos")
                sin_ch = cpool.tile([P, TCH], F32, tag="sin", name="sin")
                nc.sync.dma_start(cos_ch[:], cosF.ap()[:, tsl])
                nc.sync.dma_start(sin_ch[:], sinFpm.ap()[:, tsl])
                for ft in (hp, 4 + hp):
                    wft = apool.tile([P, 8, P], BF16, tag="wft", name="wft")
                    nc.sync.dma_start(wft[:], wqk_r[:, :, ft * P : (ft + 1) * P])
                    ps = qk_ps.tile([P, TCH], F32, tag="qkps", name="qkps")
                    for ec in range(8):
                        mm(
                            ps[:],
                            wft[:, ec, :],
                            xch[:, ec, :],
                            start=(ec == 0),
                            stop=(ec == 7),
                        )
                    dst = qkT[ft][:, tsl]
                    nc.vector.tensor_tensor(dst, ps[:], cos_ch[:], MULT)
                    bt = btpool.tile([P, TCH], BF16, tag="bt", name="bt")
                    nc.vector.tensor_tensor(bt[:], ps[:], sin_ch[:], MULT)
                    for blk in range(4):
                        a = blk * 32
                        c2 = a ^ 32  # partner half within the 64-row block
                        nc.gpsimd.dma_start(
                            dst[c2 : c2 + 32, :], bt[a : a + 32, :], accum_op=ADD
                        )

        def phase_c(hp):
            qt = qkT[hp]
            ktt = qkT[4 + hp]
            h0, h1 = 2 * hp, 2 * hp + 1
            for qci in range(n_tch):
                qsl = slice(qci * TCH, (qci + 1) * TCH)
                nkt = (TCH // P) * qci + (TCH // P)
                # both heads side by side: cols 0:512 = head h0, 512:1024 = h1
                avp = av_ps.tile([DH + 1, 2 * TCH], F32, tag="avp", name="avp")
                for ki in range(nkt):
                    ksl = slice(ki * P, (ki + 1) * P)
                    diag = ki >= (TCH // P) * qci
                    j = ki - (TCH // P) * qci
                    off = j * P if diag else 0
                    sp = s_ps.tile([P, 2 * TCH], F32, tag="sp", name="sp")
                    mm(sp[:, 0:TCH], ktt[0:64, ksl], qt[0:64, qsl], start=True, stop=True)
                    mm(
                        sp[:, TCH : 2 * TCH],
                        ktt[64:128, ksl],
                        qt[64:128, qsl],
                        start=True,
                        stop=True,
                    )
                    if diag:
                        jsl = slice(j * P, (j + 1) * P)
                        jsl2 = slice(TCH + j * P, TCH + (j + 1) * P)
                        mm(
                            sp[:, jsl],
                            ident_sb[:],
                            mtri_sb[:],
                            start=False,
                            stop=True,
                            skip_group_check=True,
                        )
                        mm(
                            sp[:, jsl2],
                            ident_sb[:],
                            mtri_sb[:],
                            start=False,
                            stop=True,
                            skip_group_check=True,
                        )
                    # one exp over both heads' live columns
                    e = epool.tile([P, 2 * TCH], BF16, tag="e", name="e")
                    sp3 = sp[:].rearrange("p (h q) -> p h q", h=2)
                    e3 = e[:].rearrange("p (h q) -> p h q", h=2)
                    nc.scalar.activation(
                        e3[:, :, off:], sp3[:, :, off:], EXP, scale=SCALE
                    )
                    mm(
                        avp[:, off:TCH],
                        vbuf[:, ki, h0, :],
                        e[:, off:TCH],
                        start=(ki == 0),
                        stop=(ki == nkt - 1),
                        skip_group_check=True,
                    )
                    mm(
                        avp[:, TCH + off : 2 * TCH],
                        vbuf[:, ki, h1, :],
                        e[:, TCH + off : 2 * TCH],
                        start=(ki == 0),
                        stop=(ki == nkt - 1),
                        skip_group_check=True,
                    )
                # denominators (row 64) -> [1, 512] f32r staging tiles
                den0 = dpool.tile([1, TCH], F32R, tag="den", name="den0")
                den1 = dpool.tile([1, TCH], F32R, tag="den", name="den1")
                nc.scalar.copy(den0[:], avp[DH : DH + 1, 0:TCH])
                nc.scalar.copy(den1[:], avp[DH : DH + 1, TCH : 2 * TCH])
                den2 = dpool.tile([2, TCH], F32R, tag="den2", name="den2")
                nc.sync.dma_start(den2[0:1, :], den0[:])
                nc.sync.dma_start(den2[1:2, :], den1[:])
                # one K=2 matmul: block-diag ones lhsT broadcasts head-0
                # denom to partitions 0-63 and head-1 to 64-127
                rb = bc_ps.tile([P, TCH], F32, tag="rb", name="rb")
                mm(rb[:], ones2_sb[:, :], den2[:], start=True, stop=True)
                rec = rpool.tile([P, TCH], F32, tag="rec", name="rec")
                rscr = rpool.tile([P, TCH], F32, tag="rscr", name="rscr")
                nc.vector.reciprocal_approx_accurate(rec[:], rb[:], rscr[:])
                # fused normalize + PSUM->SBUF drain of y^T
                nc.vector.tensor_tensor(
                    yT[hp][0:64, qsl], avp[0:DH, 0:TCH], rec[0:64, :], MULT
                )
                nc.vector.tensor_tensor(
                    yT[hp][64:128, qsl],
                    avp[0:DH, TCH : 2 * TCH],
                    rec[64:128, :],
                    MULT,
                )

        phase_a(0)
        phase_a(1)
        phase_c(0)
        phase_a(2)
        phase_c(1)
        phase_a(3)
        phase_c(2)
        phase_c(3)
        ac.close()

        # ---------- phase D: output projection ----------
        with ExitStack() as d:
            dppool = d.enter_context(tc.tile_pool(name="dproj", bufs=1))
            opool = d.enter_context(tc.tile_pool(name="outsb", bufs=3))
            o_ps = d.enter_context(tc.tile_pool(name="o_psum", bufs=2, space="PSUM"))

            wo_sb = dppool.tile([P, 4, D], BF16, tag="wo", name="wo")
            nc.sync.dma_start(wo_sb[:], woT.ap().rearrange("(co p) j -> p co j", p=P))
            for tti in range(S // P):
                tsl = slice(tti * P, (tti + 1) * P)
                for jc in range(2):
                    jsl = slice(jc * TCH, (jc + 1) * TCH)
                    op = o_ps.tile([P, TCH], F32, tag="op", name="op")
                    for cc in range(4):
                        mm(
                            op[:],
                            yT[cc][:, tsl],
                            wo_sb[:, cc, jsl],
                            start=(cc == 0),
                            stop=(cc == 3),
                        )
                    ot = opool.tile([P, TCH], F32, tag="ot", name="ot")
                    nc.vector.tensor_copy(ot[:], op[:])
                    nc.sync.dma_start(outp.ap()[tsl, jsl], ot[:])


def _build(S=S_FULL):
    key = ("nc", S)
    if key in _CACHE:
        return _CACHE[key]
    nc = bacc.Bacc("TRN2", target_bir_lowering=False, debug=False, num_devices=8)
    xT = nc.dram_tensor("xT", [D, S], BF16, kind="ExternalInput")
    wqkT = nc.dram_tensor("wqkT", [D, FQK], BF16, kind="ExternalInput")
    wvT = nc.dram_tensor("wvT", [D, FV], BF16, kind="ExternalInput")
    woT = nc.dram_tensor("woT", [FV, D], BF16, kind="ExternalInput")
    cosF = nc.dram_tensor("cosF", [P, S], F32, kind="ExternalInput")
    sinFpm = nc.dram_tensor("sinFpm", [P, S], F32, kind="ExternalInput")
    mtri = nc.dram_tensor("mtri", [P, P], BF16, kind="ExternalInput")
    ident = nc.dram_tensor("ident", [P, P], BF16, kind="ExternalInput")
    ones2 = nc.dram_tensor("ones2", [2, P], F32R, kind="ExternalInput")
    onesv = nc.dram_tensor(
        "onesv", [P, S // P, NH_CORE, 1], BF16, kind="ExternalInput"
    )
    outp = nc.dram_tensor("outp", [S, D], F32, kind="ExternalOutput")
    with tile.TileContext(nc) as tc:
        _emit(nc, tc, S, xT, wqkT, wvT, woT, cosF, sinFpm, mtri, ident, ones2, onesv, outp)
    nc.compile()
    _CACHE[key] = nc
    return nc


def host_inputs(x, wqkv, wo, token_positions, S=S_FULL):
    """Build the 8 per-core input maps (host-side sharding / layout prep)."""
    x = np.asarray(x, dtype=np.float32)
    wqkv = np.asarray(wqkv, dtype=np.float32)
    wo = np.asarray(wo, dtype=np.float32)
    pos = np.asarray(token_positions).astype(np.float32)

    d_model = x.shape[2]
    wq, wk, wv = wqkv[0:d_model], wqkv[d_model : 2 * d_model], wqkv[2 * d_model :]

    inv = np.float32(ROPE_THETA) ** (
        -np.arange(0, DH, 2, dtype=np.float32) / np.float32(DH)
    )  # [32]
    ang = pos[None, :] * inv[:, None]  # [32, S]
    cos32 = np.cos(ang).astype(np.float32)
    sin32 = np.sin(ang).astype(np.float32)
    cosF = np.tile(cos32, (4, 1))  # [128, S]
    sinFpm = np.tile(np.concatenate([sin32, -sin32], axis=0), (2, 1))  # [128, S]

    import ml_dtypes

    a = np.arange(P)
    mtri = np.where(a[:, None] > a[None, :], np.float32(NEG), np.float32(0.0))
    mtri = mtri.astype(ml_dtypes.bfloat16)
    ident = np.eye(P, dtype=ml_dtypes.bfloat16)
    S = x.shape[1]
    ones2 = np.zeros((2, P), np.float32)
    ones2[0, 0:64] = 1.0
    ones2[1, 64:128] = 1.0
    onesv = np.ones((P, S // P, NH_CORE, 1), ml_dtypes.bfloat16)

    perm64 = np.concatenate([np.arange(0, DH, 2), np.arange(1, DH, 2)])

    in_maps = []
    for ci in range(8):
        bi, hg = divmod(ci, 2)
        xT = np.ascontiguousarray(x[bi].T)
        rows = []
        for blk in (wq, wk):
            for h in range(hg * NH_CORE, (hg + 1) * NH_CORE):
                rows.append(blk[h * DH : (h + 1) * DH][perm64])
        wqkT = np.ascontiguousarray(np.concatenate(rows, axis=0).T)
        wvT = np.ascontiguousarray(wv[hg * FV : (hg + 1) * FV].T)
        woT = np.ascontiguousarray(wo[:, hg * FV : (hg + 1) * FV].T)
        xT = xT.astype(ml_dtypes.bfloat16)
        wqkT = wqkT.astype(ml_dtypes.bfloat16)
        wvT = wvT.astype(ml_dtypes.bfloat16)
        woT = woT.astype(ml_dtypes.bfloat16)
        in_maps.append(
            {
                "xT": xT,
                "wqkT": wqkT,
                "wvT": wvT,
                "woT": woT,
                "cosF": cosF,
                "sinFpm": sinFpm,
                "mtri": mtri,
                "ident": ident,
                "ones2": ones2,
                "onesv": onesv,
            }
        )
    return in_maps


def _install_ntff_hook():
    """Recreate the antenv.axon_hooks NTFF profile hook this image lacks
    (same ctypes shim trn_agent_boot would register). Dev/profiling only."""
    import contextlib
    import ctypes
    import os
    import types

    try:
        import antenv.axon_hooks  # noqa: F401

        return
    except ImportError:
        pass
    so_path = "/opt/axon/libaxon_pjrt.so"
    if not os.path.exists(so_path):
        return
    lib = ctypes.CDLL(so_path)
    if not hasattr(lib, "axon_start_nrt_profile"):
        return
    lib.axon_start_nrt_profile.argtypes = [
        ctypes.POINTER(ctypes.c_int64),
        ctypes.c_size_t,
    ]
    lib.axon_start_nrt_profile.restype = ctypes.c_int64
    lib.axon_stop_nrt_profile.argtypes = [ctypes.c_char_p]
    lib.axon_stop_nrt_profile.restype = ctypes.c_int64

    @contextlib.contextmanager
    def _hook(output_dir, device_ids):
        import jax

        jax.devices()
        if device_ids:
            ids = (ctypes.c_int64 * len(device_ids))(*device_ids)
            rc = lib.axon_start_nrt_profile(ids, len(device_ids))
        else:
            rc = lib.axon_start_nrt_profile(None, 0)
        if rc != 0:
            raise RuntimeError(f"axon_start_nrt_profile rc={rc}")
        try:
            yield
        finally:
            n = lib.axon_stop_nrt_profile(str(output_dir).encode())
            if n < 0:
                raise RuntimeError(f"axon_stop_nrt_profile rc={n}")

    import antenv
    from concourse import bass_utils as _bu

    _bu.upload_artifacts = lambda d: d  # no bucket access in this container
    mod = types.ModuleType("antenv.axon_hooks")
    mod.get_axon_ntff_profile_hook = lambda: _hook
    mod.set_axon_ntff_profile_hook = lambda h: None
    sys.modules["antenv.axon_hooks"] = mod
    antenv.axon_hooks = mod


def kernel(x, wqkv, wo, token_positions, trace=False):
    if trace:
        _install_ntff_hook()
    nc = _build()
    in_maps = host_inputs(x, wqkv, wo, token_positions)
    res = run_bass_kernel_spmd(nc, in_maps, core_ids=list(range(8)), trace=trace)
    parts = [res.results[ci]["outp"] for ci in range(8)]
    out = np.stack([parts[2 * bi] + parts[2 * bi + 1] for bi in range(B)], axis=0)
    if trace:
        kernel.last_result = res
    return out
